# revision 1
# baseline (speedup 1.0000x reference)
"""Trainium2 Bass kernel for a dense transformer decoder layer.

Shapes (hardcoded): B=4, S=2048, D=1024, H=16, HD=64, F=4096, fp32.

Sharding over 8 NeuronCores: core c handles batch b=c//2 and head-half
hh=c%2 (8 of 16 heads, Megatron-style).  Per-head QKV + causal attention
+ the output-projection partial run per core; one ReduceScatter over
core pairs {2b, 2b+1} sums the two head-halves and hands each core its
own 1024-token half; each core then finishes residual + LN1 +
full-weight FFN + LN2 for those tokens.  Host assembles the 8 [D, 1024]
transposed output slices.

Everything on-chip lives in "T-layout" ([feature-on-partitions, tokens])
so no transposes are needed anywhere.  Matmuls run in float32r (~11-bit
mantissa fp32 that streams at bf16 speed); the second FFN matmul runs in
bf16 to halve SBUF for the activation buffer.  LayerNorm stats
(partition-dim sums) and [1,N]->[128,N] broadcasts are ones-matmuls on
the PE.  Softmax runs max-free (scores are O(+-10); exp is safe in
fp32); denominators come from an appended ones-column in V (M=65 pv
matmuls); causality is handled block-wise with 4 host-built diagonal
masks.
"""

import sys

sys.path.insert(0, "/opt/trn_rl_repo")

import numpy as np
import ml_dtypes

import concourse.bass as bass
import concourse.tile as tile
from concourse import bacc, mybir
from concourse.bass import ts, ds
from concourse.bass_utils import run_bass_kernel_spmd

F32 = mybir.dt.float32
F32R = mybir.dt.float32r
BF16 = mybir.dt.bfloat16
AF = mybir.ActivationFunctionType
OP = mybir.AluOpType

B, S, D, H, F = 4, 2048, 1024, 16, 4096
HD = 64
P = 128
KD = D // P  # 8 d-tiles
SB = S // P  # 16 s-blocks
SC = S // 512  # 4 s-chunks (attention)
XC_W = 256  # x streaming chunk width (stage A)
XC_N = S // XC_W
FT = F // P  # 32 f-tiles
TOK = 1024  # tokens owned per core
NC_N = 8
LN_EPS = 1e-5
AR_GROUPS = [[0, 1], [2, 3], [4, 5], [6, 7]]

# ppvec column map (per-partition vectors packed into one [P, 80] tile)
PP_BO, PP_G1, PP_BE1, PP_B2, PP_G2, PP_BE2, PP_B1 = 0, 8, 16, 24, 32, 40, 48


def round_f32r(x: np.ndarray) -> np.ndarray:
    """Round fp32 to the fp32r grid (sign+8exp+11mant in top 20 bits, RNE)."""
    b = np.ascontiguousarray(x, dtype=np.float32).view(np.uint32).astype(np.uint64)
    b = (b + 0x7FF + ((b >> 12) & 1)) & 0xFFFFF000
    return b.astype(np.uint32).view(np.float32)


def build_nc(ar_bypass: bool = False):
    nc = bacc.Bacc("TRN2", target_bir_lowering=False, num_devices=NC_N)

    def din(name, shape, dt=F32R):
        return nc.dram_tensor(name, list(shape), dt, kind="ExternalInput").ap()

    # weight layouts are partition-major on host so every DMA is contiguous
    xT = din("xT", [KD, P, S])  # x[b].T, d-tiled
    x_resid = din("x_resid", [KD, P, TOK], F32)  # exact x slice for residual
    wq = din("wq", [4, P, KD, P])  # [pair, r, d, 2*64], pre-scaled 1/sqrt(HD)
    wk = din("wk", [4, P, KD, P])
    wv = din("wv", [P, KD, 512])
    wo = din("wo", [KD, P, 4, P])  # [m, r, k'-pair, c]
    w1 = din("w1", [FT, P, KD, P], BF16)  # [f, r, d, c]
    w2 = din("w2", [KD, P, FT, P], BF16)  # [m, r, f, c]
    bqk = din("bqk", [P, 8], F32)  # cols 0-3: bq per pair, 4-7: bk per pair
    bv_row = din("bv_row", [1, 512], F32)
    ppvec = din("ppvec", [P, 80], F32)  # bo,g1,be1,b2,g2,be2 (8 each), b1 (32)
    masks = din("masks", [P, 4, 512], BF16)  # causal diag-block masks
    vones = din("vones", [P, SB, 8, 1], BF16)  # ones column for v_aug
    ones2 = din("ones2", [P, 2], F32R)  # LN stats lhsT (M=2)
    ones_row = din("ones_row", [1, P], F32)  # K=1 broadcast lhsT, fp32
    salt = din("salt", [1, 7], F32)  # unique-shape input: avoids stale-cache signature collisions

    out = nc.dram_tensor("out", [KD, P, TOK], F32, kind="ExternalOutput").ap()

    # two ReduceScatters (chunks 0-1, then 2-3) so the first hides under
    # attention compute of the later chunks; each hands the pair-core its
    # own 512-token shard of that half
    ar_in_a = nc.dram_tensor("ar_in_a", [2, D, 512], BF16).ap()
    ar_in_b = nc.dram_tensor("ar_in_b", [2, D, 512], BF16).ap()
    ar_out_a = nc.dram_tensor("ar_out_a", [D, 512], BF16).ap()
    ar_out_b = nc.dram_tensor("ar_out_b", [D, 512], BF16).ap()

    with tile.TileContext(nc) as tc:
        with (
            tc.tile_pool(name="qkv", bufs=1) as qkv_pool,
            tc.tile_pool(name="consts", bufs=1) as consts,
        ):
            # ---- resident constants ----------------------------------------
            mask_sb = consts.tile([P, 4, 512], BF16, name="mask_sb")
            nc.sync.dma_start(out=mask_sb[:], in_=masks[:])
            ones_row_sb = consts.tile([1, P], F32, name="ones_row_sb")
            nc.sync.dma_start(out=ones_row_sb[:], in_=ones_row[:])
            bv_bcast = consts.tile([P, 512], F32, name="bv_bcast")
            nc.sync.dma_start(out=bv_bcast[:], in_=bv_row[:].partition_broadcast(P))
            bqk_sb = consts.tile([P, 8], F32, name="bqk_sb")
            nc.sync.dma_start(out=bqk_sb[:], in_=bqk[:])
            salt_sb = consts.tile([1, 7], F32, name="salt_sb")
            nc.sync.dma_start(out=salt_sb[:], in_=salt[:])

            # ---- stage A: q/k/v projections (x streamed per 256-token chunk)
            qT = [qkv_pool.tile([P, S], F32R, tag=f"qT{p}", name=f"qT{p}") for p in range(4)]
            kT = [qkv_pool.tile([P, S], F32R, tag=f"kT{p}", name=f"kT{p}") for p in range(4)]
            v_one = qkv_pool.tile([P, SB, 8, 65], BF16, name="v_one")
            nc.sync.dma_start(out=v_one[:, :, :, 64:65], in_=vones[:])

            with (
                tc.tile_pool(name="wqk", bufs=1) as wqk_pool,
                tc.tile_pool(name="xchunk", bufs=2) as xch_pool,
                tc.tile_pool(name="ps_qkv", bufs=2, space="PSUM") as ps_qkv,
            ):
                wv_sb = wqk_pool.tile([P, KD, 512], F32R, name="wv_sb")
                nc.sync.dma_start(out=wv_sb[:], in_=wv[:])
                wq_t = [wqk_pool.tile([P, KD, P], F32R, tag=f"wq{p}", name=f"wq{p}") for p in range(4)]
                wk_t = [wqk_pool.tile([P, KD, P], F32R, tag=f"wk{p}", name=f"wk{p}") for p in range(4)]
                for hp in range(4):
                    nc.sync.dma_start(out=wq_t[hp][:], in_=wq[hp])
                    nc.sync.dma_start(out=wk_t[hp][:], in_=wk[hp])
                for n in range(XC_N):
                    xc = xch_pool.tile([P, KD, XC_W], F32R, tag="xc", name="xc")
                    for d in range(KD):
                        nc.sync.dma_start(out=xc[:, d], in_=xT[d][:, ts(n, XC_W)])
                    for hp in range(4):
                        pq = ps_qkv.tile([P, XC_W], F32, tag="pq", name="pq")
                        pk = ps_qkv.tile([P, XC_W], F32, tag="pk", name="pk")
                        for d in range(KD):
                            nc.tensor.matmul(
                                pq[:], lhsT=wq_t[hp][:, d], rhs=xc[:, d],
                                start=(d == 0), stop=(d == KD - 1),
                            )
                        for d in range(KD):
                            nc.tensor.matmul(
                                pk[:], lhsT=wk_t[hp][:, d], rhs=xc[:, d],
                                start=(d == 0), stop=(d == KD - 1),
                            )
                        nc.vector.tensor_scalar_add(
                            qT[hp][:, ts(n, XC_W)], pq[:], bqk_sb[:, hp : hp + 1]
                        )
                        nc.vector.tensor_scalar_add(
                            kT[hp][:, ts(n, XC_W)], pk[:], bqk_sb[:, 4 + hp : 5 + hp]
                        )
                    for sblk in range(XC_W // P):
                        sb = (XC_W // P) * n + sblk
                        pv = ps_qkv.tile([P, 512], F32, tag="pv", name="pv")
                        for d in range(KD):
                            nc.tensor.matmul(
                                pv[:], lhsT=xc[:, d, ts(sblk, P)], rhs=wv_sb[:, d],
                                start=(d == 0), stop=(d == KD - 1),
                            )
                        nc.vector.scalar_tensor_tensor(
                            v_one[:, sb, :, 0:64],
                            pv[:].rearrange("p (h e) -> p h e", h=8),
                            1.0,
                            bv_bcast[:].rearrange("p (h e) -> p h e", h=8),
                            OP.mult,
                            OP.add,
                        )

            # ---- stage B: attention;  stage C: output-projection partial ----
            with (
                tc.tile_pool(name="attn_p", bufs=2) as attn_pool,
                tc.tile_pool(name="probs", bufs=6) as probs_pool,
                tc.tile_pool(name="norm", bufs=2) as norm_pool,
                tc.tile_pool(name="wo_p", bufs=2) as wo_pool,
                tc.tile_pool(name="arbuf", bufs=3) as ar_pool,
                tc.tile_pool(name="ps_sc", bufs=2, space="PSUM") as ps_sc,
                tc.tile_pool(name="ps_at", bufs=2, space="PSUM") as ps_at,
                tc.tile_pool(name="ps_bc", bufs=1, space="PSUM") as ps_bc,
                tc.tile_pool(name="ps_wo", bufs=1, space="PSUM") as ps_wo,
            ):
                for n in range(SC):
                    nblk = 4 * (n + 1)
                    attn_n = attn_pool.tile([P, 4, 512], F32R, tag="attn_n", name="attn_n")
                    for hp in range(4):
                        for e in range(2):
                            h = hp * 2 + e
                            pa = ps_at.tile([65, 512], F32, tag="pa", name="pa")
                            for j2 in range(nblk // 2):
                                j0 = 2 * j2
                                psc = ps_sc.tile([P, 2, 512], F32, tag="psc", name="psc")
                                for dj in range(2):
                                    nc.tensor.matmul(
                                        psc[:, dj],
                                        lhsT=kT[hp][ds(64 * e, 64), ts(j0 + dj, P)],
                                        rhs=qT[hp][ds(64 * e, 64), ts(n, 512)],
                                        start=True, stop=True,
                                    )
                                pr = probs_pool.tile([P, 2, 512], BF16, tag="pr", name="pr")
                                nc.scalar.activation(pr[:], psc[:], AF.Exp)
                                for dj in range(2):
                                    j = j0 + dj
                                    if j // 4 == n:
                                        nc.vector.tensor_tensor(
                                            pr[:, dj], pr[:, dj], mask_sb[:, j % 4], OP.mult
                                        )
                                    nc.tensor.matmul(
                                        pa[:], lhsT=v_one[:, j, h], rhs=pr[:, dj],
                                        start=(j == 0), stop=(j == nblk - 1),
                                    )
                            ssum = norm_pool.tile([1, 512], F32, tag="ssum", name="ssum")
                            nc.scalar.copy(ssum[:], pa[64:65, :])
                            recip = norm_pool.tile([1, 512], F32, tag="recip", name="recip")
                            nc.vector.reciprocal_approx_fast(recip[:], ssum[:])
                            pbc = ps_bc.tile([64, 512], F32, tag="pbc", name="pbc")
                            nc.tensor.matmul(
                                pbc[:], lhsT=ones_row_sb[:, 0:64], rhs=recip[:],
                                start=True, stop=True,
                            )
                            bc_sb = norm_pool.tile([64, 512], F32, tag="bc_sb", name="bc_sb")
                            nc.scalar.copy(bc_sb[:], pbc[:])
                            if e == 0:
                                nc.vector.tensor_tensor(
                                    attn_n[0:64, hp], pa[0:64, :], bc_sb[:], OP.mult
                                )
                            else:
                                tmp = norm_pool.tile([64, 512], F32R, tag="tmp1", name="tmp1")
                                nc.vector.tensor_tensor(
                                    tmp[:], pa[0:64, :], bc_sb[:], OP.mult
                                )
                                nc.sync.dma_start(
                                    out=attn_n[ds(64, 64), hp], in_=tmp[:]
                                )
                    # output projection partial for this token chunk
                    for m in range(KD):
                        wo_t = wo_pool.tile([P, 4, P], F32R, tag="wo", name="wo_t")
                        nc.sync.dma_start(out=wo_t[:], in_=wo[m])
                        pw = ps_wo.tile([P, 512], F32, tag="pw", name="pw")
                        for kp in range(4):
                            nc.tensor.matmul(
                                pw[:], lhsT=wo_t[:, kp], rhs=attn_n[:, kp],
                                start=(kp == 0), stop=(kp == 3),
                            )
                        arb = ar_pool.tile([P, 512], BF16, tag="arb", name="arb")
                        nc.vector.tensor_copy(arb[:], pw[:])
                        ar_tgt = ar_in_a if n < 2 else ar_in_b
                        nc.sync.dma_start(
                            out=ar_tgt[n % 2, ds(m * P, P), :], in_=arb[:]
                        )
                    if n == 1 or n == 3:
                        ar_i, ar_o = (ar_in_a, ar_out_a) if n == 1 else (ar_in_b, ar_out_b)
                        if ar_bypass:
                            nc.sync.dma_start(out=ar_o[:], in_=ar_i[0])
                        else:
                            nc.gpsimd.collective_compute(
                                "ReduceScatter",
                                OP.add,
                                replica_groups=AR_GROUPS,
                                ins=[ar_i[:]],
                                outs=[ar_o[:]],
                            )

        # ---- stage D: residual + LN1;  stage E: FFN;  LN2; output ---------
        with (
            tc.tile_pool(name="post", bufs=1) as post,
            tc.tile_pool(name="consts2", bufs=1) as consts2,
        ):
            ones2_sb2 = consts2.tile([P, 2], F32R, name="ones2_sb2")
            nc.sync.dma_start(out=ones2_sb2[:], in_=ones2[:])
            ones_row_sb2 = consts2.tile([1, P], F32, name="ones_row_sb2")
            nc.sync.dma_start(out=ones_row_sb2[:], in_=ones_row[:])
            eps_t2 = consts2.tile([1, 1], F32, name="eps_t2")
            nc.vector.memset(eps_t2[:], LN_EPS)
            ppv = consts2.tile([P, 80], F32, name="ppv")
            nc.sync.dma_start(out=ppv[:], in_=ppvec[:])

            def pp(base, i):
                return ppv[:, base + i : base + i + 1]

            def layer_norm_T(r_tiles, g_base, be_base, out_tiles, ps_pool, sqp, statp, ln_tag):
                """r_tiles: KD x [P, TOK] (mutated in place); writes out_tiles."""
                psum_s = ps_pool.tile([2, TOK], F32, tag=f"ps_s_{ln_tag}", name="psum_s")
                psum_q = ps_pool.tile([2, TOK], F32, tag=f"ps_q_{ln_tag}", name="psum_q")
                sq_tiles = []
                for m in range(KD):
                    sq = sqp.tile([P, TOK], F32R, tag="sq", name="sq")
                    nc.scalar.activation(sq[:], r_tiles[m][:], AF.Square)
                    sq_tiles.append(sq)
                for half in range(TOK // 512):
                    for m in range(KD):
                        nc.tensor.matmul(
                            psum_s[:, ts(half, 512)], lhsT=ones2_sb2[:],
                            rhs=r_tiles[m][:, ts(half, 512)],
                            start=(m == 0), stop=(m == KD - 1),
                        )
                    for m in range(KD):
                        nc.tensor.matmul(
                            psum_q[:, ts(half, 512)], lhsT=ones2_sb2[:],
                            rhs=sq_tiles[m][:, ts(half, 512)],
                            start=(m == 0), stop=(m == KD - 1),
                        )
                mean = statp.tile([1, TOK], F32, tag="mean", name="mean")
                nc.vector.tensor_scalar_mul(mean[:], psum_s[0:1, :], 1.0 / D)
                work = statp.tile([1, TOK], F32, tag="work", name="work")
                nc.vector.tensor_scalar_mul(work[:], psum_q[0:1, :], 1.0 / D)
                m2 = statp.tile([1, TOK], F32, tag="m2", name="m2")
                nc.vector.tensor_tensor(m2[:], mean[:], mean[:], OP.mult)
                nc.vector.tensor_tensor(work[:], work[:], m2[:], OP.subtract)
                nc.scalar.activation(work[:], work[:], AF.Sqrt, bias=eps_t2[:])
                rstd = statp.tile([1, TOK], F32, tag="rstd", name="rstd")
                nc.vector.reciprocal(rstd[:], work[:])
                pmb = ps_pool.tile([P, TOK], F32, tag=f"pmb_{ln_tag}", name="pmb")
                prb = ps_pool.tile([P, TOK], F32, tag=f"prb_{ln_tag}", name="prb")
                for half in range(TOK // 512):
                    nc.tensor.matmul(
                        pmb[:, ts(half, 512)], lhsT=ones_row_sb2[:],
                        rhs=mean[:, ts(half, 512)], start=True, stop=True,
                    )
                    nc.tensor.matmul(
                        prb[:, ts(half, 512)], lhsT=ones_row_sb2[:],
                        rhs=rstd[:, ts(half, 512)], start=True, stop=True,
                    )
                for m in range(KD):
                    nc.vector.tensor_tensor(
                        r_tiles[m][:], r_tiles[m][:], pmb[:], OP.subtract
                    )
                    nc.vector.tensor_tensor(
                        r_tiles[m][:], r_tiles[m][:], prb[:], OP.mult
                    )
                    nc.vector.scalar_tensor_tensor(
                        out_tiles[m][:],
                        r_tiles[m][:],
                        pp(g_base, m),
                        pp(be_base, m).to_broadcast((P, TOK)),
                        OP.mult,
                        OP.add,
                    )

            r1 = [post.tile([P, TOK], F32R, tag=f"r1_{m}", name=f"r1_{m}") for m in range(KD)]
            h1 = [post.tile([P, TOK], F32R, tag=f"h1_{m}", name=f"h1_{m}") for m in range(KD)]
            with (
                tc.tile_pool(name="ln1_ps", bufs=1, space="PSUM") as ln1_ps,
                tc.tile_pool(name="ln1_sq", bufs=2) as ln1_sq,
                tc.tile_pool(name="ln1_st", bufs=1) as ln1_st,
                tc.tile_pool(name="arload", bufs=3) as arload,
            ):
                for m in range(KD):
                    art = arload.tile([P, TOK], BF16, tag="art", name="art")
                    nc.sync.dma_start(out=art[:, 0:512], in_=ar_out_a[ds(m * P, P), :])
                    nc.sync.dma_start(out=art[:, 512:1024], in_=ar_out_b[ds(m * P, P), :])
                    xr = arload.tile([P, TOK], F32, tag="xr", name="xr")
                    nc.sync.dma_start(out=xr[:], in_=x_resid[m])
                    nc.vector.scalar_tensor_tensor(
                        r1[m][:], art[:], pp(PP_BO, m), xr[:], OP.add, OP.add
                    )
                layer_norm_T(r1, PP_G1, PP_BE1, h1, ln1_ps, ln1_sq, ln1_st, "ln1")

            # FFN (full weights, own 1024 tokens)
            r2 = r1  # reuse r1 tiles as the pre-LN2 residual buffers
            with (
                tc.tile_pool(name="gbuf", bufs=1) as gbuf,
                tc.tile_pool(name="w1s", bufs=3) as w1s,
                tc.tile_pool(name="w2s", bufs=3) as w2s,
                tc.tile_pool(name="ps_ff", bufs=2, space="PSUM") as ps_ff,
            ):
                gT = gbuf.tile([P, FT, TOK], BF16, name="gT")
                h1b = [gbuf.tile([P, TOK], BF16, tag=f"h1b_{d}", name=f"h1b_{d}") for d in range(KD)]
                for d in range(KD):
                    nc.vector.tensor_copy(h1b[d][:], h1[d][:])
                for f in range(FT):
                    w1_t = w1s.tile([P, KD, P], BF16, tag="w1", name="w1_t")
                    nc.sync.dma_start(out=w1_t[:], in_=w1[f])
                    pg = ps_ff.tile([P, TOK], F32, tag="pg", name="pg")
                    for half in range(TOK // 512):
                        for d in range(KD):
                            nc.tensor.matmul(
                                pg[:, ts(half, 512)],
                                lhsT=w1_t[:, d], rhs=h1b[d][:, ts(half, 512)],
                                start=(d == 0), stop=(d == KD - 1),
                            )
                    nc.scalar.activation(gT[:, f], pg[:], AF.Gelu, bias=pp(PP_B1, f))
                for m in range(KD):
                    w2_t = w2s.tile([P, FT, P], BF16, tag="w2", name="w2_t")
                    nc.sync.dma_start(out=w2_t[:], in_=w2[m])
                    p2 = ps_ff.tile([P, TOK], F32, tag="p2", name="p2")
                    for half in range(TOK // 512):
                        for f in range(FT):
                            nc.tensor.matmul(
                                p2[:, ts(half, 512)],
                                lhsT=w2_t[:, f], rhs=gT[:, f, ts(half, 512)],
                                start=(f == 0), stop=(f == FT - 1),
                            )
                    nc.vector.tensor_scalar_add(p2[:], p2[:], pp(PP_B2, m))
                    nc.vector.tensor_tensor(r2[m][:], p2[:], h1[m][:], OP.add)

            out_tiles = [post.tile([P, TOK], F32, tag=f"o_{m}", name=f"o_{m}") for m in range(KD)]
            with (
                tc.tile_pool(name="ln2_ps", bufs=1, space="PSUM") as ln2_ps,
                tc.tile_pool(name="ln2_sq", bufs=2) as ln2_sq,
                tc.tile_pool(name="ln2_st", bufs=1) as ln2_st,
            ):
                layer_norm_T(r2, PP_G2, PP_BE2, out_tiles, ln2_ps, ln2_sq, ln2_st, "ln2")
            for m in range(KD):
                nc.sync.dma_start(out=out[m], in_=out_tiles[m][:])

    nc.compile()
    return nc


def shard_inputs(x, Wq, bq_, Wk, bk_, Wv, bv_, Wo, bo, W1, b1, W2, b2, g1, be1, g2, be2):
    """Build the per-core in_maps (all numpy, host-side)."""
    x = np.asarray(x, np.float32)
    Wq = np.asarray(Wq, np.float32) / np.sqrt(HD)
    Wk = np.asarray(Wk, np.float32)
    Wv = np.asarray(Wv, np.float32)
    Wo = np.asarray(Wo, np.float32)
    W1 = np.asarray(W1, np.float32)
    W2 = np.asarray(W2, np.float32)

    # shared, core-independent tensors
    w1_t = np.ascontiguousarray(
        W1.reshape(KD, P, FT, P).transpose(2, 1, 0, 3)
    ).astype(ml_dtypes.bfloat16)  # w1[f, r, d, c] = W1[d*128+r, f*128+c]
    w2_t = np.ascontiguousarray(
        W2.reshape(FT, P, KD, P).transpose(2, 1, 0, 3)
    ).astype(ml_dtypes.bfloat16)  # w2[m, r, f, c] = W2[f*128+r, m*128+c]

    ppvec = np.zeros((P, 80), np.float32)
    for base, vec in [
        (PP_BO, bo), (PP_G1, g1), (PP_BE1, be1), (PP_B2, b2), (PP_G2, g2), (PP_BE2, be2),
    ]:
        ppvec[:, base : base + KD] = np.asarray(vec, np.float32).reshape(KD, P).T
    ppvec[:, PP_B1 : PP_B1 + FT] = np.asarray(b1, np.float32).reshape(FT, P).T

    iota = np.arange(512)
    masks = np.zeros((4, P, 512), np.float32)
    for jj in range(4):
        masks[jj] = (iota[None, :] >= (P * jj + np.arange(P))[:, None]).astype(np.float32)
    masks = np.ascontiguousarray(masks.transpose(1, 0, 2))  # [P, 4, 512]
    vones = np.ones((P, SB, 8, 1), ml_dtypes.bfloat16)
    ones2 = np.ones((P, 2), np.float32)
    ones_row = np.ones((1, P), np.float32)

    in_maps = []
    for c in range(NC_N):
        b_i, hh = c // 2, c % 2
        heads = slice(hh * 8, hh * 8 + 8)
        xT_c = round_f32r(x[b_i].T.reshape(KD, P, S))
        own = np.r_[hh * 512 : hh * 512 + 512, 1024 + hh * 512 : 1024 + hh * 512 + 512]
        x_resid_c = np.ascontiguousarray(x[b_i][own].T.reshape(KD, P, TOK))

        Wq8 = Wq[heads].reshape(8, KD, P, HD)  # [h, d, r, e]
        Wk8 = Wk[heads].reshape(8, KD, P, HD)
        Wv8 = Wv[heads]  # [8, D, HD]
        wq_c = np.empty((4, P, KD, P), np.float32)
        wk_c = np.empty((4, P, KD, P), np.float32)
        for p_i in range(4):
            for e in range(2):
                h = 2 * p_i + e
                wq_c[p_i, :, :, e * 64 : (e + 1) * 64] = Wq8[h].transpose(1, 0, 2)
                wk_c[p_i, :, :, e * 64 : (e + 1) * 64] = Wk8[h].transpose(1, 0, 2)
        wv_c = np.ascontiguousarray(
            Wv8.reshape(8, KD, P, HD).transpose(2, 1, 0, 3).reshape(P, KD, 8 * HD)
        )  # wv[r, d, h*64+e] = Wv8[h, d*128+r, e]
        Wo_own = Wo[hh * 512 : (hh + 1) * 512]  # [512, D]
        wo_c = np.ascontiguousarray(
            Wo_own.reshape(4, P, KD, P).transpose(2, 1, 0, 3)
        )  # wo[m, r, kp, c] = Wo_own[kp*128+r, m*128+c]

        bq8 = np.asarray(bq_, np.float32)[heads].reshape(4, P)
        bk8 = np.asarray(bk_, np.float32)[heads].reshape(4, P)
        bqk_c = np.concatenate([bq8.T, bk8.T], axis=1)  # [P, 8]
        bv8 = np.asarray(bv_, np.float32)[heads]

        in_maps.append(
            {
                "xT": xT_c,
                "x_resid": x_resid_c,
                "wq": round_f32r(wq_c),
                "wk": round_f32r(wk_c),
                "wv": round_f32r(wv_c),
                "wo": round_f32r(wo_c),
                "w1": w1_t,
                "w2": w2_t,
                "bqk": bqk_c,
                "bv_row": bv8.reshape(1, 8 * HD),
                "ppvec": ppvec,
                "masks": masks.astype(ml_dtypes.bfloat16),
                "vones": vones,
                "ones2": ones2,
                "ones_row": ones_row,
                "salt": np.full((1, 7), 7.0, np.float32),
            }
        )
    return in_maps


_NC_CACHE = {}


def _get_nc(ar_bypass=False):
    key = bool(ar_bypass)
    if key not in _NC_CACHE:
        _NC_CACHE[key] = build_nc(ar_bypass)
    return _NC_CACHE[key]


def assemble(results):
    out = np.empty((B, S, D), np.float32)
    for c in range(NC_N):
        b_i, hh = c // 2, c % 2
        own = np.r_[hh * 512 : hh * 512 + 512, 1024 + hh * 512 : 1024 + hh * 512 + 512]
        oT = results[c]["out"].reshape(D, TOK)
        out[b_i, own, :] = oT.T
    return out


def kernel(**inputs) -> np.ndarray:
    nc = _get_nc()
    in_maps = shard_inputs(
        inputs["x"], inputs["Wq"], inputs["bq"], inputs["Wk"], inputs["bk"],
        inputs["Wv"], inputs["bv"], inputs["Wo"], inputs["bo"],
        inputs["W1"], inputs["b1"], inputs["W2"], inputs["b2"],
        inputs["g1"], inputs["be1"], inputs["g2"], inputs["be2"],
    )
    res = run_bass_kernel_spmd(nc, in_maps, list(range(NC_N)))
    return assemble(res.results)



# revision 14
# speedup vs baseline: 1.2031x; 1.2031x over previous
"""Trainium2 Bass kernel for a dense transformer decoder layer.

Shapes (hardcoded): B=4, S=2048, D=1024, H=16, HD=64, F=4096, fp32.

Sharding over 8 NeuronCores: core c handles batch b=c//2 and head-half
hh=c%2 (8 of 16 heads, Megatron-style).  Per-head QKV + causal attention
+ the output-projection partial run per core; one ReduceScatter over
core pairs {2b, 2b+1} sums the two head-halves and hands each core its
own 1024-token half; each core then finishes residual + LN1 +
full-weight FFN + LN2 for those tokens.

This version fuses QKV projection, attention, and the output projection
into one software-pipelined phase: QKV chunks c+1 and the WO matmuls of
chunk c-1 are interleaved into attention chunk c's matmul stream as
dense PE filler, so the tensor engine never idles long enough for the
HAM clock gate to re-throttle it to 1.2 GHz (that throttling cost the
previous version ~400us at half clock).  Scores for the two heads of a
pair are row-packed (contraction rows 0-63 / 64-127, explicit
tile_position) so they run concurrently in the PE array.  Softmax is
max-free; the denominator rides as a 65th V column; its reciprocal is
broadcast across partitions via a DRAM round-trip DMA instead of a PE
ones-matmul (saves PSUM banks).  LN stats run as ones-matmuls; the
mean/rstd broadcasts also use the DRAM bounce.  LN1+FFN run in two
512-token halves so FFN1 on the first half hides the second
ReduceScatter; LN2 stats are interleaved into the FFN2 m-loop.
"""

import sys
from contextlib import ExitStack

sys.path.insert(0, "/opt/trn_rl_repo")

import numpy as np
import ml_dtypes

import concourse.bass as bass
import concourse.tile as tile
from concourse import bacc, mybir
from concourse.bass import ts, ds
from concourse.bass_utils import run_bass_kernel_spmd

F32 = mybir.dt.float32
F32R = mybir.dt.float32r
BF16 = mybir.dt.bfloat16
AF = mybir.ActivationFunctionType
OP = mybir.AluOpType

B, S, D, H, F = 4, 2048, 1024, 16, 4096
HD = 64
P = 128
KD = D // P  # 8 d-tiles
SB = S // P  # 16 key-blocks
SC = S // 512  # 4 chunks of 512 tokens
FT = F // P  # 32 f-tiles
TOK = 1024  # tokens owned per core
NC_N = 8
LN_EPS = 1e-5
AR_GROUPS = [[0, 1], [2, 3], [4, 5], [6, 7]]

# ppvec column map (per-partition vectors packed into one [P, 80] tile)
PP_BO, PP_G1, PP_BE1, PP_B2, PP_G2, PP_BE2, PP_B1 = 0, 8, 16, 24, 32, 40, 48


def round_f32r(x: np.ndarray) -> np.ndarray:
    """Round fp32 to the fp32r grid (sign+8exp+11mant in top 20 bits, RNE)."""
    b = np.ascontiguousarray(x, dtype=np.float32).view(np.uint32).astype(np.uint64)
    b = (b + 0x7FF + ((b >> 12) & 1)) & 0xFFFFF000
    return b.astype(np.uint32).view(np.float32)


def build_nc(ar_bypass: bool = False):
    nc = bacc.Bacc("TRN2", target_bir_lowering=False, num_devices=NC_N)

    def din(name, shape, dt=F32R):
        return nc.dram_tensor(name, list(shape), dt, kind="ExternalInput").ap()

    xT = din("xT", [KD, P, S])  # x[b].T, d-tiled
    x_resid = din("x_resid", [KD, P, TOK], F32)  # exact x slice for residual
    wq = din("wq", [4, P, KD, P])  # [pair, r, d, 2*64], pre-scaled 1/sqrt(HD)
    wk = din("wk", [4, P, KD, P])
    wv = din("wv", [P, KD, 512])
    wo = din("wo", [KD, P, 4, P])  # [m, r, k'-pair, c]
    w1 = din("w1", [FT, P, KD, P], BF16)  # [f, r, d, c]
    w2 = din("w2", [KD, P, FT, P], BF16)  # [m, r, f, c]
    bqk = din("bqk", [P, 8], F32)  # cols 0-3: bq per pair, 4-7: bk per pair
    bv_row = din("bv_row", [1, 512], F32)
    ppvec = din("ppvec", [P, 80], F32)  # bo,g1,be1,b2,g2,be2 (8 each), b1 (32)
    masks = din("masks", [P, 4, 512], BF16)  # causal diag-block masks
    vones = din("vones", [P, SB, 8, 1], BF16)  # ones column for v_aug
    ones2 = din("ones2", [P, 2], F32R)  # LN stats lhsT (M=2)
    salt = din("salt", [1, 7], F32)  # unique-shape input: cache signature salt

    out = nc.dram_tensor("out", [KD, P, TOK], F32, kind="ExternalOutput").ap()

    # pair ReduceScatters (chunks 0-1 then 2-3)
    ar_in_a = nc.dram_tensor("ar_in_a", [2, D, 512], BF16).ap()
    ar_in_b = nc.dram_tensor("ar_in_b", [2, D, 512], BF16).ap()
    ar_out_a = nc.dram_tensor("ar_out_a", [D, 512], BF16).ap()
    ar_out_b = nc.dram_tensor("ar_out_b", [D, 512], BF16).ap()

    # DRAM scratch rows for partition-broadcast bounces
    rb_sc = nc.dram_tensor("rb_sc", [SC, 4, 1, 1024], F32).ap()  # softmax recips
    ln_sc = nc.dram_tensor("ln_sc", [6, TOK], F32).ap()  # ln1 m/r halves, ln2 m/r

    with tile.TileContext(nc) as tc:
        attn_ctx = ExitStack()
        with tc.tile_pool(name="consts", bufs=1) as consts, attn_ctx:
            kv_pool = attn_ctx.enter_context(tc.tile_pool(name="kv", bufs=1))
            qc_pool = attn_ctx.enter_context(tc.tile_pool(name="qc", bufs=2))
            attn_pool = attn_ctx.enter_context(tc.tile_pool(name="attn_n", bufs=2))
            probs_pool = attn_ctx.enter_context(tc.tile_pool(name="probs", bufs=2))
            pas_pool = attn_ctx.enter_context(tc.tile_pool(name="pas", bufs=2))
            bc_pool = attn_ctx.enter_context(tc.tile_pool(name="bcast", bufs=2))
            small_pool = attn_ctx.enter_context(tc.tile_pool(name="small", bufs=1))
            wo_pool = attn_ctx.enter_context(tc.tile_pool(name="wo_p", bufs=2))
            ar_pool = attn_ctx.enter_context(tc.tile_pool(name="arbuf", bufs=2))
            ps_sc = attn_ctx.enter_context(
                tc.tile_pool(name="ps_sc", bufs=2, space="PSUM")
            )
            ps_at = attn_ctx.enter_context(
                tc.tile_pool(name="ps_at", bufs=1, space="PSUM")
            )

            # ---- resident constants ----------------------------------------
            mask_sb = consts.tile([P, 4, 512], BF16, name="mask_sb")
            nc.sync.dma_start(out=mask_sb[:], in_=masks[:])
            bv_bcast = consts.tile([P, 512], F32, name="bv_bcast")
            nc.sync.dma_start(out=bv_bcast[:], in_=bv_row[:].partition_broadcast(P))
            bqk_sb = consts.tile([P, 8], F32, name="bqk_sb")
            nc.sync.dma_start(out=bqk_sb[:], in_=bqk[:])
            ones2_sb = consts.tile([P, 2], F32R, name="ones2_sb")
            nc.sync.dma_start(out=ones2_sb[:], in_=ones2[:])
            ones2_bf = consts.tile([P, 2], BF16, name="ones2_bf")
            nc.vector.memset(ones2_bf[:], 1.0)
            eps_t = consts.tile([1, 1], F32, name="eps_t")
            nc.vector.memset(eps_t[:], LN_EPS)
            ppv = consts.tile([P, 80], F32, name="ppv")
            nc.sync.dma_start(out=ppv[:], in_=ppvec[:])
            salt_sb = consts.tile([1, 7], F32, name="salt_sb")
            nc.sync.dma_start(out=salt_sb[:], in_=salt[:])

            def pp(base, i):
                return ppv[:, base + i : base + i + 1]

            # persistent attention tensors
            kT = [kv_pool.tile([P, S], F32R, tag=f"kT{p}", name=f"kT{p}") for p in range(4)]
            v_one = kv_pool.tile([P, SB, 8, 65], BF16, name="v_one")
            nc.sync.dma_start(out=v_one[:, :, :, 64:65], in_=vones[:])

            # ================= fused QKV + attention + WO =================
            # qkv pools open last (close first): strict pool stack order
            qkv_ctx = ExitStack()
            wqk_pool = qkv_ctx.enter_context(tc.tile_pool(name="wqk", bufs=1))
            xch_pool = qkv_ctx.enter_context(tc.tile_pool(name="xchunk", bufs=2))
            ps_qkv = qkv_ctx.enter_context(
                tc.tile_pool(name="ps_qkv", bufs=2, space="PSUM")
            )

            wq_t = [wqk_pool.tile([P, KD, P], F32R, tag=f"wq{p}", name=f"wq{p}") for p in range(4)]
            wk_t = [wqk_pool.tile([P, KD, P], F32R, tag=f"wk{p}", name=f"wk{p}") for p in range(4)]
            wv_sb = wqk_pool.tile([P, KD, 512], F32R, name="wv_sb")

            qc_tiles = {}
            attn_tiles = {}

            def emit_qkv(c):
                """Generator: one yield per PE accumulation group (12)."""
                xc = xch_pool.tile([P, KD, 512], F32R, tag="xc", name="xc")
                for d in range(KD):
                    nc.sync.dma_start(out=xc[:, d], in_=xT[d][:, ts(c, 512)])
                qc = qc_pool.tile([P, 4, 512], F32R, tag="qc", name="qc")
                qc_tiles[c] = qc
                for hp in range(4):
                    pq = ps_qkv.tile([P, 512], F32, tag="pqkv", name="pq")
                    for d in range(KD):
                        nc.tensor.matmul(
                            pq[:], lhsT=wq_t[hp][:, d], rhs=xc[:, d],
                            start=(d == 0), stop=(d == KD - 1),
                        )
                    nc.vector.tensor_scalar_add(
                        qc[:, hp], pq[:], bqk_sb[:, hp : hp + 1]
                    )
                    yield
                    pk = ps_qkv.tile([P, 512], F32, tag="pqkv", name="pk")
                    for d in range(KD):
                        nc.tensor.matmul(
                            pk[:], lhsT=wk_t[hp][:, d], rhs=xc[:, d],
                            start=(d == 0), stop=(d == KD - 1),
                        )
                    nc.vector.tensor_scalar_add(
                        kT[hp][:, ts(c, 512)], pk[:], bqk_sb[:, 4 + hp : 5 + hp]
                    )
                    yield
                for sblk in range(4):
                    sb = 4 * c + sblk
                    pv = ps_qkv.tile([P, 512], F32, tag="pqkv", name="pv")
                    for d in range(KD):
                        nc.tensor.matmul(
                            pv[:], lhsT=xc[:, d, ts(sblk, P)], rhs=wv_sb[:, d],
                            start=(d == 0), stop=(d == KD - 1),
                        )
                    nc.vector.scalar_tensor_tensor(
                        v_one[:, sb, :, 0:64],
                        pv[:].rearrange("p (h e) -> p h e", h=8),
                        1.0,
                        bv_bcast[:].rearrange("p (h e) -> p h e", h=8),
                        OP.mult,
                        OP.add,
                    )
                    yield

            def emit_wo(c):
                """Generator: one yield per WO m-tile (8). Writes ar_in."""
                attn_n = attn_tiles[c]
                for m in range(KD):
                    wo_t = wo_pool.tile([P, 4, P], F32R, tag="wo", name="wo_t")
                    nc.sync.dma_start(out=wo_t[:], in_=wo[m])
                    pw = ps_sc.tile([P, 2, 512], F32, tag="psc", name="pw")
                    for kp in range(4):
                        nc.tensor.matmul(
                            pw[:, 0], lhsT=wo_t[:, kp], rhs=attn_n[:, kp],
                            start=(kp == 0), stop=(kp == 3),
                        )
                    arb = ar_pool.tile([P, 512], BF16, tag="arb", name="arb")
                    nc.vector.tensor_copy(arb[:], pw[:, 0])
                    ar_tgt = ar_in_a if c < 2 else ar_in_b
                    nc.sync.dma_start(
                        out=ar_tgt[c % 2, ds(m * P, P), :], in_=arb[:]
                    )
                    yield

            # prefetch: only hp0's q/k weights + the x chunk ahead of the
            # first matmul group; stream the rest behind it
            nc.sync.dma_start(out=wq_t[0][:], in_=wq[0])
            nc.sync.dma_start(out=wk_t[0][:], in_=wk[0])
            with nc.named_scope("qkv0"):
                gen0 = emit_qkv(0)
                next(gen0)
                for hp in range(1, 4):
                    nc.sync.dma_start(out=wq_t[hp][:], in_=wq[hp])
                    nc.sync.dma_start(out=wk_t[hp][:], in_=wk[hp])
                nc.sync.dma_start(out=wv_sb[:], in_=wv[:])
                for _ in gen0:
                    pass

            for c in range(SC):
                nblk = 4 * (c + 1)
                fillers = []
                if c >= 1:
                    fillers.append(emit_wo(c - 1))
                if c + 1 < SC:
                    fillers.append(emit_qkv(c + 1))
                n_yield = {0: 12, 1: 20, 2: 20, 3: 8}[c]
                total_j = 4 * nblk
                stride = max(1, total_j // max(1, n_yield))

                def pull_filler():
                    while fillers:
                        try:
                            next(fillers[0])
                            return
                        except StopIteration:
                            fillers.pop(0)

                with nc.named_scope(f"attn{c}"):
                    qc = qc_tiles[c]
                    attn_n = attn_pool.tile(
                        [P, 4, 512], F32R, tag="attn_n", name="attn_n"
                    )
                    attn_tiles[c] = attn_n
                    jj = 0
                    for hp in range(4):
                        pa = ps_at.tile([65, 2, 512], F32, tag="pa", name="pa")
                        pr_prev = None
                        for j in range(nblk):
                            psc = ps_sc.tile([P, 2, 512], F32, tag="psc", name="psc")
                            nc.tensor.matmul(
                                psc[:, 0],
                                lhsT=kT[hp][0:64, ts(j, P)],
                                rhs=qc[0:64, hp],
                                start=True, stop=True,
                                tile_position=(0, 0),
                            )
                            nc.tensor.matmul(
                                psc[:, 1],
                                lhsT=kT[hp][ds(64, 64), ts(j, P)],
                                rhs=qc[ds(64, 64), hp],
                                start=True, stop=True,
                                tile_position=(64, 0),
                            )
                            pr = probs_pool.tile([P, 2, 512], BF16, tag="pr", name="pr")
                            nc.scalar.activation(pr[:], psc[:], AF.Exp)
                            if j // 4 == c:
                                for e in range(2):
                                    nc.vector.tensor_tensor(
                                        pr[:, e], pr[:, e], mask_sb[:, j % 4], OP.mult
                                    )
                            # lagged PV so exp(j-1) is done when PV hits PE head
                            if pr_prev is not None:
                                jp = j - 1
                                for e in range(2):
                                    nc.tensor.matmul(
                                        pa[:, e],
                                        lhsT=v_one[:, jp, 2 * hp + e],
                                        rhs=pr_prev[:, e],
                                        start=(jp == 0), stop=False,
                                    )
                            pr_prev = pr
                            jj += 1
                            if jj % stride == 0:
                                pull_filler()
                        for e in range(2):
                            nc.tensor.matmul(
                                pa[:, e],
                                lhsT=v_one[:, nblk - 1, 2 * hp + e],
                                rhs=pr_prev[:, e],
                                start=(nblk == 1), stop=True,
                            )
                        # softmax normalize; stage pa out to SBUF so the PSUM
                        # slot frees without waiting on the DRAM bounce
                        ssum = small_pool.tile([1, 2, 512], F32, tag="ssum", name="ssum")
                        nc.vector.tensor_copy(ssum[:], pa[64:65, :, :])
                        pa_s = pas_pool.tile([64, 2, 512], F32R, tag="pa_s", name="pa_s")
                        nc.vector.tensor_copy(pa_s[:], pa[0:64, :, :])
                        recip = small_pool.tile([1, 2, 512], F32, tag="recip", name="recip")
                        nc.vector.reciprocal_approx_fast(recip[:], ssum[:])
                        nc.sync.dma_start(out=rb_sc[c, hp], in_=recip[:])
                        bc = bc_pool.tile([64, 2, 512], F32, tag="bc", name="bc")
                        nc.sync.dma_start(
                            out=bc[:, 0],
                            in_=rb_sc[c, hp, :, 0:512].partition_broadcast(64),
                        )
                        nc.sync.dma_start(
                            out=bc[:, 1],
                            in_=rb_sc[c, hp, :, 512:1024].partition_broadcast(64),
                        )
                        nc.vector.tensor_tensor(
                            attn_n[0:64, hp], pa_s[:, 0, :], bc[:, 0], OP.mult
                        )
                        tmp = small_pool.tile([64, 512], F32R, tag="tmp1", name="tmp1")
                        nc.vector.tensor_tensor(
                            tmp[:], pa_s[:, 1, :], bc[:, 1], OP.mult
                        )
                        nc.sync.dma_start(out=attn_n[ds(64, 64), hp], in_=tmp[:])
                        pull_filler()
                # drain remaining fillers before moving on
                while fillers:
                    pull_filler()
                if c == 2:
                    # WO(1) finished inside attn(2) fillers: launch RS-a
                    if ar_bypass:
                        nc.sync.dma_start(out=ar_out_a[:], in_=ar_in_a[0])
                    else:
                        nc.gpsimd.collective_compute(
                            "ReduceScatter",
                            OP.add,
                            replica_groups=AR_GROUPS,
                            ins=[ar_in_a[:]],
                            outs=[ar_out_a[:]],
                        )
                    # qkv weights / x-chunks / qkv psum no longer needed
                    qkv_ctx.close()

            # WO(3) + RS-b
            with nc.named_scope("wo3"):
                for _ in emit_wo(3):
                    pass
            if ar_bypass:
                nc.sync.dma_start(out=ar_out_b[:], in_=ar_in_b[0])
            else:
                nc.gpsimd.collective_compute(
                    "ReduceScatter",
                    OP.add,
                    replica_groups=AR_GROUPS,
                    ins=[ar_in_b[:]],
                    outs=[ar_out_b[:]],
                )
            attn_ctx.close()

            # ================= LN1 + FFN + LN2 =================
            ffn_ctx = ExitStack()
            with ffn_ctx:
                gbuf = ffn_ctx.enter_context(tc.tile_pool(name="gbuf", bufs=1))
                post = ffn_ctx.enter_context(tc.tile_pool(name="post", bufs=1))
                w1s = ffn_ctx.enter_context(tc.tile_pool(name="w1s", bufs=3))
                w2s = ffn_ctx.enter_context(tc.tile_pool(name="w2s", bufs=2))
                lnop = ffn_ctx.enter_context(tc.tile_pool(name="lnop", bufs=2))
                lnbc = ffn_ctx.enter_context(tc.tile_pool(name="lnbc", bufs=2))
                sqp = ffn_ctx.enter_context(tc.tile_pool(name="sqp", bufs=4))
                ps_ff = ffn_ctx.enter_context(
                    tc.tile_pool(name="ps_ff", bufs=2, space="PSUM")
                )
                ps_f2 = ffn_ctx.enter_context(
                    tc.tile_pool(name="ps_f2", bufs=2, space="PSUM")
                )
                ps_ln2 = ffn_ctx.enter_context(
                    tc.tile_pool(name="ps_ln2", bufs=2, space="PSUM")
                )
                outp = ffn_ctx.enter_context(tc.tile_pool(name="outp", bufs=2))

                gT = gbuf.tile([P, FT, TOK], BF16, name="gT")
                r1 = [post.tile([P, TOK], F32R, tag=f"r1_{m}", name=f"r1_{m}") for m in range(KD)]
                h1b = [post.tile([P, TOK], BF16, tag=f"h1b_{m}", name=f"h1b_{m}") for m in range(KD)]

                def ln1_half(half, ar_src, sc_m, sc_r):
                    """Residual add + LN1 for one 512-token half -> h1b."""
                    lo = half * 512
                    for m in range(KD):
                        art = lnop.tile([P, 512], BF16, tag="art", name="art")
                        nc.sync.dma_start(out=art[:], in_=ar_src[ds(m * P, P), :])
                        xr = lnop.tile([P, 512], F32, tag="xr", name="xr")
                        nc.sync.dma_start(out=xr[:], in_=x_resid[m][:, lo : lo + 512])
                        nc.vector.scalar_tensor_tensor(
                            r1[m][:, lo : lo + 512], art[:], pp(PP_BO, m),
                            xr[:], OP.add, OP.add,
                        )
                    pss = ps_ln2.tile([2, 2, 512], F32, tag="lnps2", name="pss1")
                    for m in range(KD):
                        nc.tensor.matmul(
                            pss[:, 0], lhsT=ones2_sb[:], rhs=r1[m][:, lo : lo + 512],
                            start=(m == 0), stop=(m == KD - 1),
                        )
                    for w in range(2):  # two 4-tile waves cap sq liveness
                        sq = [None] * 4
                        for i in range(4):
                            m = 4 * w + i
                            sq[i] = sqp.tile([P, 512], BF16, tag="sq", name="sq")
                            nc.vector.tensor_tensor(
                                sq[i][:], r1[m][:, lo : lo + 512],
                                r1[m][:, lo : lo + 512], OP.mult,
                            )
                        for i in range(4):
                            m = 4 * w + i
                            nc.tensor.matmul(
                                pss[:, 1], lhsT=ones2_bf[:], rhs=sq[i][:],
                                start=(m == 0), stop=(m == KD - 1),
                            )
                    mean = lnop.tile([1, 512], F32, tag="lnm", name="lnm")
                    nc.vector.tensor_scalar_mul(mean[:], pss[0:1, 0, :], 1.0 / D)
                    nc.sync.dma_start(out=ln_sc[sc_m : sc_m + 1, 0:512], in_=mean[:])
                    var = lnop.tile([1, 512], F32, tag="lnv", name="lnv")
                    nc.vector.tensor_scalar_mul(var[:], pss[0:1, 1, :], 1.0 / D)
                    m2 = lnop.tile([1, 512], F32, tag="lnm2", name="lnm2")
                    nc.vector.tensor_tensor(m2[:], mean[:], mean[:], OP.mult)
                    nc.vector.tensor_tensor(var[:], var[:], m2[:], OP.subtract)
                    nc.scalar.activation(var[:], var[:], AF.Sqrt, bias=eps_t[:])
                    rstd = lnop.tile([1, 512], F32, tag="lnr", name="lnr")
                    nc.vector.reciprocal_approx_fast(rstd[:], var[:])
                    nc.sync.dma_start(out=ln_sc[sc_r : sc_r + 1, 0:512], in_=rstd[:])
                    mb = lnbc.tile([P, 512], F32, tag="lnb1", name="lnmb")
                    nc.sync.dma_start(
                        out=mb[:], in_=ln_sc[sc_m : sc_m + 1, 0:512].partition_broadcast(P)
                    )
                    rb = lnbc.tile([P, 512], F32, tag="lnb1", name="lnrb")
                    nc.sync.dma_start(
                        out=rb[:], in_=ln_sc[sc_r : sc_r + 1, 0:512].partition_broadcast(P)
                    )
                    for m in range(KD):
                        nc.vector.tensor_tensor(
                            r1[m][:, lo : lo + 512], r1[m][:, lo : lo + 512],
                            mb[:], OP.subtract,
                        )
                        nc.vector.tensor_tensor(
                            r1[m][:, lo : lo + 512], r1[m][:, lo : lo + 512],
                            rb[:], OP.mult,
                        )
                        nc.vector.scalar_tensor_tensor(
                            h1b[m][:, lo : lo + 512],
                            r1[m][:, lo : lo + 512],
                            pp(PP_G1, m),
                            pp(PP_BE1, m).to_broadcast((P, 512)),
                            OP.mult,
                            OP.add,
                        )

                def ffn1_half(half):
                    for f in range(FT):
                        w1_t = w1s.tile([P, KD, P], BF16, tag="w1", name="w1_t")
                        nc.sync.dma_start(out=w1_t[:], in_=w1[f])
                        pg = ps_ff.tile([P, 512], F32, tag="pg", name="pg")
                        for d in range(KD):
                            nc.tensor.matmul(
                                pg[:],
                                lhsT=w1_t[:, d], rhs=h1b[d][:, ts(half, 512)],
                                start=(d == 0), stop=(d == KD - 1),
                            )
                        nc.scalar.activation(
                            gT[:, f, ts(half, 512)], pg[:], AF.Gelu, bias=pp(PP_B1, f)
                        )

                with nc.named_scope("ln1_h0"):
                    ln1_half(0, ar_out_a, 0, 1)
                with nc.named_scope("ffn1_h0"):
                    ffn1_half(0)
                with nc.named_scope("ln1_h1"):
                    ln1_half(1, ar_out_b, 2, 3)
                with nc.named_scope("ffn1_h1"):
                    ffn1_half(1)

                # ---- FFN2 m-loop with LN2 stats interleaved ----
                r2 = r1  # reuse as pre-LN2 residual buffers
                ps2s = ps_ln2.tile([2, 2, 512], F32, tag="lnps2", name="ps2s")
                ps2q = ps_ln2.tile([2, 2, 512], F32, tag="lnps2", name="ps2q")
                with nc.named_scope("ffn2"):
                    for m in range(KD):
                        w2_t = w2s.tile([P, FT, P], BF16, tag="w2", name="w2_t")
                        nc.sync.dma_start(out=w2_t[:], in_=w2[m])
                        for half in range(2):
                            p2 = ps_f2.tile([P, 512], F32, tag="p2", name="p2")
                            for f in range(FT):
                                nc.tensor.matmul(
                                    p2[:],
                                    lhsT=w2_t[:, f], rhs=gT[:, f, ts(half, 512)],
                                    start=(f == 0), stop=(f == FT - 1),
                                )
                            nc.vector.scalar_tensor_tensor(
                                r2[m][:, ts(half, 512)], p2[:], pp(PP_B2, m),
                                h1b[m][:, ts(half, 512)], OP.add, OP.add,
                            )
                            # LN2 stats, incremental over m
                            nc.tensor.matmul(
                                ps2s[:, half], lhsT=ones2_sb[:],
                                rhs=r2[m][:, ts(half, 512)],
                                start=(m == 0), stop=(m == KD - 1),
                            )
                            sq2 = lnop.tile([P, 512], BF16, tag="sq2", name="sq2")
                            nc.vector.tensor_tensor(
                                sq2[:], r2[m][:, ts(half, 512)],
                                r2[m][:, ts(half, 512)], OP.mult,
                            )
                            nc.tensor.matmul(
                                ps2q[:, half], lhsT=ones2_bf[:], rhs=sq2[:],
                                start=(m == 0), stop=(m == KD - 1),
                            )

                # ---- LN2 finalize + output ----
                with nc.named_scope("ln2_out"):
                    mean2 = lnop.tile([1, TOK], F32, tag="lnm", name="ln2m")
                    var2 = lnop.tile([1, TOK], F32, tag="lnv", name="ln2v")
                    for half in range(2):
                        nc.vector.tensor_scalar_mul(
                            mean2[:, ts(half, 512)], ps2s[0:1, half, :], 1.0 / D
                        )
                        nc.vector.tensor_scalar_mul(
                            var2[:, ts(half, 512)], ps2q[0:1, half, :], 1.0 / D
                        )
                    m22 = lnop.tile([1, TOK], F32, tag="lnm2", name="ln2m2")
                    nc.vector.tensor_tensor(m22[:], mean2[:], mean2[:], OP.mult)
                    nc.vector.tensor_tensor(var2[:], var2[:], m22[:], OP.subtract)
                    nc.scalar.activation(var2[:], var2[:], AF.Sqrt, bias=eps_t[:])
                    rstd2 = lnop.tile([1, TOK], F32, tag="lnr", name="ln2r")
                    nc.vector.reciprocal_approx_fast(rstd2[:], var2[:])
                    nc.sync.dma_start(out=ln_sc[4:5, :], in_=mean2[:])
                    nc.sync.dma_start(out=ln_sc[5:6, :], in_=rstd2[:])
                    mb2 = lnbc.tile([P, TOK], F32, tag="lnb2", name="ln2mb")
                    nc.sync.dma_start(
                        out=mb2[:], in_=ln_sc[4:5, :].partition_broadcast(P)
                    )
                    rb2 = lnbc.tile([P, TOK], F32, tag="lnb2", name="ln2rb")
                    nc.sync.dma_start(
                        out=rb2[:], in_=ln_sc[5:6, :].partition_broadcast(P)
                    )
                    for m in range(KD):
                        nc.vector.tensor_tensor(r2[m][:], r2[m][:], mb2[:], OP.subtract)
                        nc.vector.tensor_tensor(r2[m][:], r2[m][:], rb2[:], OP.mult)
                        for half in range(2):
                            ot = outp.tile([P, 512], F32, tag="ot", name="ot")
                            nc.vector.scalar_tensor_tensor(
                                ot[:],
                                r2[m][:, ts(half, 512)],
                                pp(PP_G2, m),
                                pp(PP_BE2, m).to_broadcast((P, 512)),
                                OP.mult,
                                OP.add,
                            )
                            nc.sync.dma_start(
                                out=out[m][:, ts(half, 512)], in_=ot[:]
                            )

    nc.compile()
    return nc


def shard_inputs(x, Wq, bq_, Wk, bk_, Wv, bv_, Wo, bo, W1, b1, W2, b2, g1, be1, g2, be2):
    """Build the per-core in_maps (all numpy, host-side)."""
    x = np.asarray(x, np.float32)
    Wq = np.asarray(Wq, np.float32) / np.sqrt(HD)
    Wk = np.asarray(Wk, np.float32)
    Wv = np.asarray(Wv, np.float32)
    Wo = np.asarray(Wo, np.float32)
    W1 = np.asarray(W1, np.float32)
    W2 = np.asarray(W2, np.float32)

    # shared, core-independent tensors
    w1_t = np.ascontiguousarray(
        W1.reshape(KD, P, FT, P).transpose(2, 1, 0, 3)
    ).astype(ml_dtypes.bfloat16)  # w1[f, r, d, c] = W1[d*128+r, f*128+c]
    w2_t = np.ascontiguousarray(
        W2.reshape(FT, P, KD, P).transpose(2, 1, 0, 3)
    ).astype(ml_dtypes.bfloat16)  # w2[m, r, f, c] = W2[f*128+r, m*128+c]

    ppvec = np.zeros((P, 80), np.float32)
    for base, vec in [
        (PP_BO, bo), (PP_G1, g1), (PP_BE1, be1), (PP_B2, b2), (PP_G2, g2), (PP_BE2, be2),
    ]:
        ppvec[:, base : base + KD] = np.asarray(vec, np.float32).reshape(KD, P).T
    ppvec[:, PP_B1 : PP_B1 + FT] = np.asarray(b1, np.float32).reshape(FT, P).T

    iota = np.arange(512)
    masks = np.zeros((4, P, 512), np.float32)
    for jj in range(4):
        masks[jj] = (iota[None, :] >= (P * jj + np.arange(P))[:, None]).astype(np.float32)
    masks = np.ascontiguousarray(masks.transpose(1, 0, 2))  # [P, 4, 512]
    vones = np.ones((P, SB, 8, 1), ml_dtypes.bfloat16)
    ones2 = np.ones((P, 2), np.float32)

    in_maps = []
    for c in range(NC_N):
        b_i, hh = c // 2, c % 2
        heads = slice(hh * 8, hh * 8 + 8)
        xT_c = round_f32r(x[b_i].T.reshape(KD, P, S))
        own = np.r_[hh * 512 : hh * 512 + 512, 1024 + hh * 512 : 1024 + hh * 512 + 512]
        x_resid_c = np.ascontiguousarray(x[b_i][own].T.reshape(KD, P, TOK))

        Wq8 = Wq[heads].reshape(8, KD, P, HD)  # [h, d, r, e]
        Wk8 = Wk[heads].reshape(8, KD, P, HD)
        Wv8 = Wv[heads]  # [8, D, HD]
        wq_c = np.empty((4, P, KD, P), np.float32)
        wk_c = np.empty((4, P, KD, P), np.float32)
        for p_i in range(4):
            for e in range(2):
                h = 2 * p_i + e
                wq_c[p_i, :, :, e * 64 : (e + 1) * 64] = Wq8[h].transpose(1, 0, 2)
                wk_c[p_i, :, :, e * 64 : (e + 1) * 64] = Wk8[h].transpose(1, 0, 2)
        wv_c = np.ascontiguousarray(
            Wv8.reshape(8, KD, P, HD).transpose(2, 1, 0, 3).reshape(P, KD, 8 * HD)
        )  # wv[r, d, h*64+e] = Wv8[h, d*128+r, e]
        Wo_own = Wo[hh * 512 : (hh + 1) * 512]  # [512, D]
        wo_c = np.ascontiguousarray(
            Wo_own.reshape(4, P, KD, P).transpose(2, 1, 0, 3)
        )  # wo[m, r, kp, c] = Wo_own[kp*128+r, m*128+c]

        bq8 = np.asarray(bq_, np.float32)[heads].reshape(4, P)
        bk8 = np.asarray(bk_, np.float32)[heads].reshape(4, P)
        bqk_c = np.concatenate([bq8.T, bk8.T], axis=1)  # [P, 8]
        bv8 = np.asarray(bv_, np.float32)[heads]

        in_maps.append(
            {
                "xT": xT_c,
                "x_resid": x_resid_c,
                "wq": round_f32r(wq_c),
                "wk": round_f32r(wk_c),
                "wv": round_f32r(wv_c),
                "wo": round_f32r(wo_c),
                "w1": w1_t,
                "w2": w2_t,
                "bqk": bqk_c,
                "bv_row": bv8.reshape(1, 8 * HD),
                "ppvec": ppvec,
                "masks": masks.astype(ml_dtypes.bfloat16),
                "vones": vones,
                "ones2": ones2,
                "salt": np.full((1, 7), 11.0, np.float32),
            }
        )
    return in_maps


_NC_CACHE = {}


def _get_nc(ar_bypass=False):
    key = bool(ar_bypass)
    if key not in _NC_CACHE:
        _NC_CACHE[key] = build_nc(ar_bypass)
    return _NC_CACHE[key]


def assemble(results):
    out = np.empty((B, S, D), np.float32)
    for c in range(NC_N):
        b_i, hh = c // 2, c % 2
        own = np.r_[hh * 512 : hh * 512 + 512, 1024 + hh * 512 : 1024 + hh * 512 + 512]
        oT = results[c]["out"].reshape(D, TOK)
        out[b_i, own, :] = oT.T
    return out


def kernel(**inputs) -> np.ndarray:
    nc = _get_nc()
    in_maps = shard_inputs(
        inputs["x"], inputs["Wq"], inputs["bq"], inputs["Wk"], inputs["bk"],
        inputs["Wv"], inputs["bv"], inputs["Wo"], inputs["bo"],
        inputs["W1"], inputs["b1"], inputs["W2"], inputs["b2"],
        inputs["g1"], inputs["be1"], inputs["g2"], inputs["be2"],
    )
    res = run_bass_kernel_spmd(nc, in_maps, list(range(NC_N)))
    return assemble(res.results)


# revision 17
# speedup vs baseline: 1.2393x; 1.0301x over previous
"""Trainium2 Bass kernel for a dense transformer decoder layer.

Shapes (hardcoded): B=4, S=2048, D=1024, H=16, HD=64, F=4096, fp32.

Sharding over 8 NeuronCores: core c handles batch b=c//2 and head-half
hh=c%2 (8 of 16 heads, Megatron-style).  Per-head QKV + causal attention
+ the output-projection partial run per core; one ReduceScatter over
core pairs {2b, 2b+1} sums the two head-halves and hands each core its
own 1024-token half; each core then finishes residual + LN1 +
full-weight FFN + LN2 for those tokens.

This version fuses QKV projection, attention, and the output projection
into one software-pipelined phase: QKV chunks c+1 and the WO matmuls of
chunk c-1 are interleaved into attention chunk c's matmul stream as
dense PE filler, so the tensor engine never idles long enough for the
HAM clock gate to re-throttle it to 1.2 GHz (that throttling cost the
previous version ~400us at half clock).  Scores for the two heads of a
pair are row-packed (contraction rows 0-63 / 64-127, explicit
tile_position) so they run concurrently in the PE array.  Softmax is
max-free; the denominator rides as a 65th V column; its reciprocal is
broadcast across partitions via a DRAM round-trip DMA instead of a PE
ones-matmul (saves PSUM banks).  LN stats run as ones-matmuls; the
mean/rstd broadcasts also use the DRAM bounce.  LN1+FFN run in two
512-token halves so FFN1 on the first half hides the second
ReduceScatter; LN2 stats are interleaved into the FFN2 m-loop.
"""

import sys
from contextlib import ExitStack

sys.path.insert(0, "/opt/trn_rl_repo")

import numpy as np
import ml_dtypes

import concourse.bass as bass
import concourse.tile as tile
from concourse import bacc, mybir
from concourse.bass import ts, ds
from concourse.bass_utils import run_bass_kernel_spmd

F32 = mybir.dt.float32
F32R = mybir.dt.float32r
BF16 = mybir.dt.bfloat16
AF = mybir.ActivationFunctionType
OP = mybir.AluOpType

B, S, D, H, F = 4, 2048, 1024, 16, 4096
HD = 64
P = 128
KD = D // P  # 8 d-tiles
SB = S // P  # 16 key-blocks
SC = S // 512  # 4 chunks of 512 tokens
FT = F // P  # 32 f-tiles
TOK = 1024  # tokens owned per core
NC_N = 8
LN_EPS = 1e-5
AR_GROUPS = [[0, 1], [2, 3], [4, 5], [6, 7]]

# ppvec column map (per-partition vectors packed into one [P, 80] tile)
PP_BO, PP_G1, PP_BE1, PP_B2, PP_G2, PP_BE2, PP_B1 = 0, 8, 16, 24, 32, 40, 48


def round_f32r(x: np.ndarray) -> np.ndarray:
    """Round fp32 to the fp32r grid (sign+8exp+11mant in top 20 bits, RNE)."""
    b = np.ascontiguousarray(x, dtype=np.float32).view(np.uint32).astype(np.uint64)
    b = (b + 0x7FF + ((b >> 12) & 1)) & 0xFFFFF000
    return b.astype(np.uint32).view(np.float32)


def build_nc(ar_bypass: bool = False):
    nc = bacc.Bacc("TRN2", target_bir_lowering=False, num_devices=NC_N)

    def din(name, shape, dt=F32R):
        return nc.dram_tensor(name, list(shape), dt, kind="ExternalInput").ap()

    xT = din("xT", [P, KD, S])  # x[b].T, p-major so one DMA loads a chunk
    x_resid = din("x_resid", [KD, P, TOK], F32)  # exact x slice for residual
    wq = din("wq", [4, P, KD, P])  # [pair, r, d, 2*64], pre-scaled 1/sqrt(HD)
    wk = din("wk", [4, P, KD, P])
    wv = din("wv", [P, KD, 512])
    wo = din("wo", [KD, P, 4, P], BF16)  # [m, r, k'-pair, c]
    w1 = din("w1", [FT // 2, P, 2, KD, P], BF16)  # f-pairs: [fp, r, 2, d, c]
    w2 = din("w2", [KD, P, FT, P], BF16)  # [m, r, f, c]
    bqk = din("bqk", [P, 8], F32)  # cols 0-3: bq per pair, 4-7: bk per pair
    bv_row = din("bv_row", [1, 512], F32)
    ppvec = din("ppvec", [P, 80], F32)  # bo,g1,be1,b2,g2,be2 (8 each), b1 (32)
    masks = din("masks", [P, 4, 512], BF16)  # causal diag-block masks
    ones2 = din("ones2", [P, 2], F32R)  # LN stats lhsT (M=2)
    salt = din("salt", [1, 7], F32)  # unique-shape input: cache signature salt

    out = nc.dram_tensor("out", [KD, P, TOK], F32, kind="ExternalOutput").ap()

    # pair ReduceScatters (chunks 0-1 then 2-3)
    ar_in_a = nc.dram_tensor("ar_in_a", [2, D, 512], BF16).ap()
    ar_in_b = nc.dram_tensor("ar_in_b", [2, D, 512], BF16).ap()
    ar_out_a = nc.dram_tensor("ar_out_a", [D, 512], BF16).ap()
    ar_out_b = nc.dram_tensor("ar_out_b", [D, 512], BF16).ap()

    # DRAM scratch rows for partition-broadcast bounces
    rb_sc = nc.dram_tensor("rb_sc", [SC, 4, 1, 1024], F32).ap()  # softmax recips
    ln_sc = nc.dram_tensor("ln_sc", [6, TOK], F32).ap()  # ln1 m/r halves, ln2 m/r

    with tile.TileContext(nc) as tc:
        attn_ctx = ExitStack()
        with tc.tile_pool(name="consts", bufs=1) as consts, attn_ctx:
            kv_pool = attn_ctx.enter_context(tc.tile_pool(name="kv", bufs=1))
            qc_pool = attn_ctx.enter_context(tc.tile_pool(name="qc", bufs=2))
            attn_pool = attn_ctx.enter_context(tc.tile_pool(name="attn_n", bufs=2))
            probs_pool = attn_ctx.enter_context(tc.tile_pool(name="probs", bufs=2))
            pas_pool = attn_ctx.enter_context(tc.tile_pool(name="pas", bufs=2))
            bc_pool = attn_ctx.enter_context(tc.tile_pool(name="bcast", bufs=2))
            small_pool = attn_ctx.enter_context(tc.tile_pool(name="small", bufs=1))
            ar_pool = attn_ctx.enter_context(tc.tile_pool(name="arbuf", bufs=2))
            ps_sc = attn_ctx.enter_context(
                tc.tile_pool(name="ps_sc", bufs=2, space="PSUM")
            )
            ps_at = attn_ctx.enter_context(
                tc.tile_pool(name="ps_at", bufs=1, space="PSUM")
            )

            # ---- resident constants ----------------------------------------
            mask_sb = consts.tile([P, 4, 512], BF16, name="mask_sb")
            nc.sync.dma_start(out=mask_sb[:], in_=masks[:])
            bv_bcast = consts.tile([P, 512], F32, name="bv_bcast")
            nc.sync.dma_start(out=bv_bcast[:], in_=bv_row[:].partition_broadcast(P))
            bqk_sb = consts.tile([P, 8], F32, name="bqk_sb")
            nc.sync.dma_start(out=bqk_sb[:], in_=bqk[:])
            ones2_sb = consts.tile([P, 2], F32R, name="ones2_sb")
            nc.sync.dma_start(out=ones2_sb[:], in_=ones2[:])
            ones2_bf = consts.tile([P, 2], BF16, name="ones2_bf")
            nc.vector.memset(ones2_bf[:], 1.0)
            eps_t = consts.tile([1, 1], F32, name="eps_t")
            nc.vector.memset(eps_t[:], LN_EPS)
            ppv = consts.tile([P, 80], F32, name="ppv")
            nc.sync.dma_start(out=ppv[:], in_=ppvec[:])
            salt_sb = consts.tile([1, 7], F32, name="salt_sb")
            nc.sync.dma_start(out=salt_sb[:], in_=salt[:])

            def pp(base, i):
                return ppv[:, base + i : base + i + 1]

            # persistent attention tensors
            kT = [kv_pool.tile([P, S], F32R, tag=f"kT{p}", name=f"kT{p}") for p in range(4)]
            v_one = kv_pool.tile([P, SB, 8, 65], BF16, name="v_one")
            nc.vector.memset(v_one[:, :, :, 64:65], 1.0)
            wo_sb = kv_pool.tile([P, KD, 4, P], BF16, name="wo_sb")
            for m in range(KD):
                nc.sync.dma_start(out=wo_sb[:, m], in_=wo[m])

            # ================= fused QKV + attention + WO =================
            # qkv pools open last (close first): strict pool stack order
            qkv_ctx = ExitStack()
            wqk_pool = qkv_ctx.enter_context(tc.tile_pool(name="wqk", bufs=1))
            xch_pool = qkv_ctx.enter_context(tc.tile_pool(name="xchunk", bufs=2))
            ps_qkv = qkv_ctx.enter_context(
                tc.tile_pool(name="ps_qkv", bufs=2, space="PSUM")
            )

            wq_t = [wqk_pool.tile([P, KD, P], F32R, tag=f"wq{p}", name=f"wq{p}") for p in range(4)]
            wk_t = [wqk_pool.tile([P, KD, P], F32R, tag=f"wk{p}", name=f"wk{p}") for p in range(4)]
            wv_sb = wqk_pool.tile([P, KD, 512], F32R, name="wv_sb")

            qc_tiles = {}
            attn_tiles = {}

            def emit_qkv(c):
                """Generator: one yield per PE accumulation group (12)."""
                xc = xch_pool.tile([P, KD, 512], F32R, tag="xc", name="xc")
                nc.sync.dma_start(out=xc[:], in_=xT[:, :, ts(c, 512)])
                qc = qc_pool.tile([P, 4, 512], F32R, tag="qc", name="qc")
                qc_tiles[c] = qc
                for hp in range(4):
                    pq = ps_qkv.tile([P, 512], F32, tag="pqkv", name="pq")
                    for d in range(KD):
                        nc.tensor.matmul(
                            pq[:], lhsT=wq_t[hp][:, d], rhs=xc[:, d],
                            start=(d == 0), stop=(d == KD - 1),
                        )
                    nc.vector.tensor_scalar_add(
                        qc[:, hp], pq[:], bqk_sb[:, hp : hp + 1]
                    )
                    yield
                    pk = ps_qkv.tile([P, 512], F32, tag="pqkv", name="pk")
                    for d in range(KD):
                        nc.tensor.matmul(
                            pk[:], lhsT=wk_t[hp][:, d], rhs=xc[:, d],
                            start=(d == 0), stop=(d == KD - 1),
                        )
                    nc.vector.tensor_scalar_add(
                        kT[hp][:, ts(c, 512)], pk[:], bqk_sb[:, 4 + hp : 5 + hp]
                    )
                    yield
                for sblk in range(4):
                    sb = 4 * c + sblk
                    pv = ps_qkv.tile([P, 512], F32, tag="pqkv", name="pv")
                    for d in range(KD):
                        nc.tensor.matmul(
                            pv[:], lhsT=xc[:, d, ts(sblk, P)], rhs=wv_sb[:, d],
                            start=(d == 0), stop=(d == KD - 1),
                        )
                    nc.vector.scalar_tensor_tensor(
                        v_one[:, sb, :, 0:64],
                        pv[:].rearrange("p (h e) -> p h e", h=8),
                        1.0,
                        bv_bcast[:].rearrange("p (h e) -> p h e", h=8),
                        OP.mult,
                        OP.add,
                    )
                    yield

            def emit_wo(c):
                """Generator: one yield per WO m-tile (8). Writes ar_in."""
                attn_n = attn_tiles[c]
                for m in range(KD):
                    pw = ps_sc.tile([P, 2, 512], F32, tag="psc", name="pw")
                    for kp in range(4):
                        nc.tensor.matmul(
                            pw[:, 0], lhsT=wo_sb[:, m, kp], rhs=attn_n[:, kp],
                            start=(kp == 0), stop=(kp == 3),
                        )
                    arb = ar_pool.tile([P, 512], BF16, tag="arb", name="arb")
                    nc.vector.tensor_copy(arb[:], pw[:, 0])
                    ar_tgt = ar_in_a if c < 2 else ar_in_b
                    nc.sync.dma_start(
                        out=ar_tgt[c % 2, ds(m * P, P), :], in_=arb[:]
                    )
                    yield

            # prefetch: only hp0's q/k weights + the x chunk ahead of the
            # first matmul group; stream the rest behind it
            nc.sync.dma_start(out=wq_t[0][:], in_=wq[0])
            nc.sync.dma_start(out=wk_t[0][:], in_=wk[0])
            with nc.named_scope("qkv0"):
                gen0 = emit_qkv(0)
                next(gen0)
                for hp in range(1, 4):
                    nc.sync.dma_start(out=wq_t[hp][:], in_=wq[hp])
                    nc.sync.dma_start(out=wk_t[hp][:], in_=wk[hp])
                nc.sync.dma_start(out=wv_sb[:], in_=wv[:])
                for _ in gen0:
                    pass

            for c in range(SC):
                nblk = 4 * (c + 1)
                fillers = []
                if c >= 1:
                    fillers.append(emit_wo(c - 1))
                if c + 1 < SC:
                    fillers.append(emit_qkv(c + 1))
                n_yield = {0: 12, 1: 20, 2: 20, 3: 8}[c]
                total_j = 4 * nblk
                stride = max(1, total_j // max(1, n_yield))

                def pull_filler():
                    while fillers:
                        try:
                            next(fillers[0])
                            return
                        except StopIteration:
                            fillers.pop(0)

                with nc.named_scope(f"attn{c}"):
                    qc = qc_tiles[c]
                    attn_n = attn_pool.tile(
                        [P, 4, 512], BF16, tag="attn_n", name="attn_n"
                    )
                    attn_tiles[c] = attn_n
                    jj = 0
                    for hp in range(4):
                        pa = ps_at.tile([65, 2, 512], F32, tag="pa", name="pa")
                        pr_prev = None
                        for j in range(nblk):
                            psc = ps_sc.tile([P, 2, 512], F32, tag="psc", name="psc")
                            nc.tensor.matmul(
                                psc[:, 0],
                                lhsT=kT[hp][0:64, ts(j, P)],
                                rhs=qc[0:64, hp],
                                start=True, stop=True,
                                tile_position=(0, 0),
                            )
                            nc.tensor.matmul(
                                psc[:, 1],
                                lhsT=kT[hp][ds(64, 64), ts(j, P)],
                                rhs=qc[ds(64, 64), hp],
                                start=True, stop=True,
                                tile_position=(64, 0),
                            )
                            pr = probs_pool.tile([P, 2, 512], BF16, tag="pr", name="pr")
                            nc.scalar.activation(pr[:], psc[:], AF.Exp)
                            if j // 4 == c:
                                for e in range(2):
                                    nc.vector.tensor_tensor(
                                        pr[:, e], pr[:, e], mask_sb[:, j % 4], OP.mult
                                    )
                            # lagged PV so exp(j-1) is done when PV hits PE head
                            if pr_prev is not None:
                                jp = j - 1
                                for e in range(2):
                                    nc.tensor.matmul(
                                        pa[:, e],
                                        lhsT=v_one[:, jp, 2 * hp + e],
                                        rhs=pr_prev[:, e],
                                        start=(jp == 0), stop=False,
                                    )
                            pr_prev = pr
                            jj += 1
                            if jj % stride == 0:
                                pull_filler()
                        for e in range(2):
                            nc.tensor.matmul(
                                pa[:, e],
                                lhsT=v_one[:, nblk - 1, 2 * hp + e],
                                rhs=pr_prev[:, e],
                                start=(nblk == 1), stop=True,
                            )
                        # softmax normalize; stage pa out to SBUF so the PSUM
                        # slot frees without waiting on the DRAM bounce
                        ssum = small_pool.tile([1, 2, 512], F32, tag="ssum", name="ssum")
                        nc.vector.tensor_copy(ssum[:], pa[64:65, :, :])
                        pa_s = pas_pool.tile([64, 2, 512], F32R, tag="pa_s", name="pa_s")
                        nc.vector.tensor_copy(pa_s[:], pa[0:64, :, :])
                        recip = small_pool.tile([1, 2, 512], F32, tag="recip", name="recip")
                        nc.vector.reciprocal_approx_fast(recip[:], ssum[:])
                        nc.sync.dma_start(out=rb_sc[c, hp], in_=recip[:])
                        bc = bc_pool.tile([64, 2, 512], F32, tag="bc", name="bc")
                        nc.sync.dma_start(
                            out=bc[:], in_=rb_sc[c, hp].partition_broadcast(64)
                        )
                        nc.vector.tensor_tensor(
                            attn_n[0:64, hp], pa_s[:, 0, :], bc[:, 0], OP.mult
                        )
                        tmp = small_pool.tile([64, 512], BF16, tag="tmp1", name="tmp1")
                        nc.vector.tensor_tensor(
                            tmp[:], pa_s[:, 1, :], bc[:, 1], OP.mult
                        )
                        nc.sync.dma_start(out=attn_n[ds(64, 64), hp], in_=tmp[:])
                        pull_filler()
                # drain remaining fillers before moving on
                while fillers:
                    pull_filler()
                if c == 2:
                    # WO(1) finished inside attn(2) fillers: launch RS-a
                    if ar_bypass:
                        nc.sync.dma_start(out=ar_out_a[:], in_=ar_in_a[0])
                    else:
                        nc.gpsimd.collective_compute(
                            "ReduceScatter",
                            OP.add,
                            replica_groups=AR_GROUPS,
                            ins=[ar_in_a[:]],
                            outs=[ar_out_a[:]],
                        )
                    # qkv weights / x-chunks / qkv psum no longer needed
                    qkv_ctx.close()

            # WO(3) + RS-b
            with nc.named_scope("wo3"):
                for _ in emit_wo(3):
                    pass
            if ar_bypass:
                nc.sync.dma_start(out=ar_out_b[:], in_=ar_in_b[0])
            else:
                nc.gpsimd.collective_compute(
                    "ReduceScatter",
                    OP.add,
                    replica_groups=AR_GROUPS,
                    ins=[ar_in_b[:]],
                    outs=[ar_out_b[:]],
                )
            attn_ctx.close()

            # ================= LN1 + FFN + LN2 =================
            ffn_ctx = ExitStack()
            with ffn_ctx:
                gbuf = ffn_ctx.enter_context(tc.tile_pool(name="gbuf", bufs=1))
                post = ffn_ctx.enter_context(tc.tile_pool(name="post", bufs=1))
                w1s = ffn_ctx.enter_context(tc.tile_pool(name="w1s", bufs=3))
                w2s = ffn_ctx.enter_context(tc.tile_pool(name="w2s", bufs=2))
                lnop = ffn_ctx.enter_context(tc.tile_pool(name="lnop", bufs=2))
                lnst = ffn_ctx.enter_context(tc.tile_pool(name="lnst", bufs=1))
                lnio = ffn_ctx.enter_context(tc.tile_pool(name="lnio", bufs=3))
                lnbc = ffn_ctx.enter_context(tc.tile_pool(name="lnbc", bufs=2))
                sqp = ffn_ctx.enter_context(tc.tile_pool(name="sqp", bufs=4))
                ps_ff = ffn_ctx.enter_context(
                    tc.tile_pool(name="ps_ff", bufs=2, space="PSUM")
                )
                ps_f2 = ffn_ctx.enter_context(
                    tc.tile_pool(name="ps_f2", bufs=2, space="PSUM")
                )
                ps_ln2 = ffn_ctx.enter_context(
                    tc.tile_pool(name="ps_ln2", bufs=2, space="PSUM")
                )
                outp = ffn_ctx.enter_context(tc.tile_pool(name="outp", bufs=2))

                gT = gbuf.tile([P, FT, TOK], BF16, name="gT")
                r1 = [post.tile([P, TOK], F32R, tag=f"r1_{m}", name=f"r1_{m}") for m in range(KD)]
                h1b = [post.tile([P, TOK], BF16, tag=f"h1b_{m}", name=f"h1b_{m}") for m in range(KD)]

                def ln1_half(half, ar_src, sc_m, sc_r):
                    """Residual add + LN1 for one 512-token half -> h1b."""
                    lo = half * 512
                    for m in range(KD):
                        art = lnio.tile([P, 512], BF16, tag="art", name="art")
                        nc.sync.dma_start(out=art[:], in_=ar_src[ds(m * P, P), :])
                        xr = lnio.tile([P, 512], F32, tag="xr", name="xr")
                        nc.sync.dma_start(out=xr[:], in_=x_resid[m][:, lo : lo + 512])
                        nc.vector.scalar_tensor_tensor(
                            r1[m][:, lo : lo + 512], art[:], pp(PP_BO, m),
                            xr[:], OP.add, OP.add,
                        )
                    pss = ps_ln2.tile([2, 2, 512], F32, tag="lnps2", name="pss1")
                    for m in range(KD):
                        nc.tensor.matmul(
                            pss[:, 0], lhsT=ones2_sb[:], rhs=r1[m][:, lo : lo + 512],
                            start=(m == 0), stop=(m == KD - 1),
                        )
                    for w in range(2):  # two 4-tile waves cap sq liveness
                        sq = [None] * 4
                        for i in range(4):
                            m = 4 * w + i
                            sq[i] = sqp.tile([P, 512], BF16, tag="sq", name="sq")
                            nc.vector.tensor_tensor(
                                sq[i][:], r1[m][:, lo : lo + 512],
                                r1[m][:, lo : lo + 512], OP.mult,
                            )
                        for i in range(4):
                            m = 4 * w + i
                            nc.tensor.matmul(
                                pss[:, 1], lhsT=ones2_bf[:], rhs=sq[i][:],
                                start=(m == 0), stop=(m == KD - 1),
                            )
                    mean = lnst.tile([1, 512], F32, tag="lnm", name="lnm")
                    nc.vector.tensor_scalar_mul(mean[:], pss[0:1, 0, :], 1.0 / D)
                    nc.sync.dma_start(out=ln_sc[sc_m : sc_m + 1, 0:512], in_=mean[:])
                    var = lnst.tile([1, 512], F32, tag="lnv", name="lnv")
                    nc.vector.tensor_scalar_mul(var[:], pss[0:1, 1, :], 1.0 / D)
                    m2 = lnst.tile([1, 512], F32, tag="lnm2", name="lnm2")
                    nc.vector.tensor_tensor(m2[:], mean[:], mean[:], OP.mult)
                    nc.vector.tensor_tensor(var[:], var[:], m2[:], OP.subtract)
                    nc.scalar.activation(var[:], var[:], AF.Sqrt, bias=eps_t[:])
                    rstd = lnst.tile([1, 512], F32, tag="lnr", name="lnr")
                    nc.vector.reciprocal_approx_fast(rstd[:], var[:])
                    nc.sync.dma_start(out=ln_sc[sc_r : sc_r + 1, 0:512], in_=rstd[:])
                    mb = lnbc.tile([P, 512], F32, tag="lnb1", name="lnmb")
                    nc.sync.dma_start(
                        out=mb[:], in_=ln_sc[sc_m : sc_m + 1, 0:512].partition_broadcast(P)
                    )
                    rb = lnbc.tile([P, 512], F32, tag="lnb1", name="lnrb")
                    nc.sync.dma_start(
                        out=rb[:], in_=ln_sc[sc_r : sc_r + 1, 0:512].partition_broadcast(P)
                    )
                    for m in range(KD):
                        nc.vector.tensor_tensor(
                            r1[m][:, lo : lo + 512], r1[m][:, lo : lo + 512],
                            mb[:], OP.subtract,
                        )
                        nc.vector.tensor_tensor(
                            r1[m][:, lo : lo + 512], r1[m][:, lo : lo + 512],
                            rb[:], OP.mult,
                        )
                        nc.vector.scalar_tensor_tensor(
                            h1b[m][:, lo : lo + 512],
                            r1[m][:, lo : lo + 512],
                            pp(PP_G1, m),
                            pp(PP_BE1, m).to_broadcast((P, 512)),
                            OP.mult,
                            OP.add,
                        )

                def ffn1_half(half):
                    for fp in range(FT // 2):
                        w1_t = w1s.tile([P, 2, KD, P], BF16, tag="w1", name="w1_t")
                        nc.sync.dma_start(out=w1_t[:], in_=w1[fp])
                        for sub in range(2):
                            f = 2 * fp + sub
                            pg = ps_ff.tile([P, 512], F32, tag="pg", name="pg")
                            for d in range(KD):
                                nc.tensor.matmul(
                                    pg[:],
                                    lhsT=w1_t[:, sub, d], rhs=h1b[d][:, ts(half, 512)],
                                    start=(d == 0), stop=(d == KD - 1),
                                )
                            nc.scalar.activation(
                                gT[:, f, ts(half, 512)], pg[:], AF.Gelu,
                                bias=pp(PP_B1, f),
                            )

                with nc.named_scope("ln1_h0"):
                    ln1_half(0, ar_out_a, 0, 1)
                with nc.named_scope("ffn1_h0"):
                    ffn1_half(0)
                with nc.named_scope("ln1_h1"):
                    ln1_half(1, ar_out_b, 2, 3)
                with nc.named_scope("ffn1_h1"):
                    ffn1_half(1)

                # ---- FFN2 m-loop with LN2 stats interleaved ----
                r2 = r1  # reuse as pre-LN2 residual buffers
                ps2s = ps_ln2.tile([2, 2, 512], F32, tag="lnps2", name="ps2s")
                ps2q = ps_ln2.tile([2, 2, 512], F32, tag="lnps2", name="ps2q")
                with nc.named_scope("ffn2"):
                    for m in range(KD):
                        w2_t = w2s.tile([P, FT, P], BF16, tag="w2", name="w2_t")
                        nc.sync.dma_start(out=w2_t[:], in_=w2[m])
                        for half in range(2):
                            p2 = ps_f2.tile([P, 512], F32, tag="p2", name="p2")
                            for f in range(FT):
                                nc.tensor.matmul(
                                    p2[:],
                                    lhsT=w2_t[:, f], rhs=gT[:, f, ts(half, 512)],
                                    start=(f == 0), stop=(f == FT - 1),
                                )
                            nc.vector.scalar_tensor_tensor(
                                r2[m][:, ts(half, 512)], p2[:], pp(PP_B2, m),
                                h1b[m][:, ts(half, 512)], OP.add, OP.add,
                            )
                            # LN2 stats, incremental over m
                            nc.tensor.matmul(
                                ps2s[:, half], lhsT=ones2_sb[:],
                                rhs=r2[m][:, ts(half, 512)],
                                start=(m == 0), stop=(m == KD - 1),
                            )
                            sq2 = lnop.tile([P, 512], BF16, tag="sq2", name="sq2")
                            nc.vector.tensor_tensor(
                                sq2[:], r2[m][:, ts(half, 512)],
                                r2[m][:, ts(half, 512)], OP.mult,
                            )
                            nc.tensor.matmul(
                                ps2q[:, half], lhsT=ones2_bf[:], rhs=sq2[:],
                                start=(m == 0), stop=(m == KD - 1),
                            )

                # ---- LN2 finalize + output ----
                with nc.named_scope("ln2_out"):
                    mean2 = lnst.tile([1, TOK], F32, tag="lnm", name="ln2m")
                    var2 = lnst.tile([1, TOK], F32, tag="lnv", name="ln2v")
                    for half in range(2):
                        nc.vector.tensor_scalar_mul(
                            mean2[:, ts(half, 512)], ps2s[0:1, half, :], 1.0 / D
                        )
                        nc.vector.tensor_scalar_mul(
                            var2[:, ts(half, 512)], ps2q[0:1, half, :], 1.0 / D
                        )
                    m22 = lnst.tile([1, TOK], F32, tag="lnm2", name="ln2m2")
                    nc.vector.tensor_tensor(m22[:], mean2[:], mean2[:], OP.mult)
                    nc.vector.tensor_tensor(var2[:], var2[:], m22[:], OP.subtract)
                    nc.scalar.activation(var2[:], var2[:], AF.Sqrt, bias=eps_t[:])
                    rstd2 = lnst.tile([1, TOK], F32, tag="lnr", name="ln2r")
                    nc.vector.reciprocal_approx_fast(rstd2[:], var2[:])
                    nc.sync.dma_start(out=ln_sc[4:5, :], in_=mean2[:])
                    nc.sync.dma_start(out=ln_sc[5:6, :], in_=rstd2[:])
                    mb2 = lnbc.tile([P, TOK], F32, tag="lnb2", name="ln2mb")
                    nc.sync.dma_start(
                        out=mb2[:], in_=ln_sc[4:5, :].partition_broadcast(P)
                    )
                    rb2 = lnbc.tile([P, TOK], F32, tag="lnb2", name="ln2rb")
                    nc.sync.dma_start(
                        out=rb2[:], in_=ln_sc[5:6, :].partition_broadcast(P)
                    )
                    for m in range(KD):
                        nc.gpsimd.tensor_sub(r2[m][:], r2[m][:], mb2[:])
                        nc.vector.tensor_tensor(r2[m][:], r2[m][:], rb2[:], OP.mult)
                        for half in range(2):
                            ot = outp.tile([P, 512], F32, tag="ot", name="ot")
                            nc.vector.scalar_tensor_tensor(
                                ot[:],
                                r2[m][:, ts(half, 512)],
                                pp(PP_G2, m),
                                pp(PP_BE2, m).to_broadcast((P, 512)),
                                OP.mult,
                                OP.add,
                            )
                            nc.sync.dma_start(
                                out=out[m][:, ts(half, 512)], in_=ot[:]
                            )

    nc.compile()
    return nc


def shard_inputs(x, Wq, bq_, Wk, bk_, Wv, bv_, Wo, bo, W1, b1, W2, b2, g1, be1, g2, be2):
    """Build the per-core in_maps (all numpy, host-side)."""
    x = np.asarray(x, np.float32)
    Wq = np.asarray(Wq, np.float32) / np.sqrt(HD)
    Wk = np.asarray(Wk, np.float32)
    Wv = np.asarray(Wv, np.float32)
    Wo = np.asarray(Wo, np.float32)
    W1 = np.asarray(W1, np.float32)
    W2 = np.asarray(W2, np.float32)

    # shared, core-independent tensors
    w1_t = np.ascontiguousarray(
        W1.reshape(KD, P, FT, P).transpose(2, 1, 0, 3)
    ).astype(ml_dtypes.bfloat16)  # w1[f, r, d, c] = W1[d*128+r, f*128+c]
    w1_t = np.ascontiguousarray(
        w1_t.reshape(FT // 2, 2, P, KD, P).transpose(0, 2, 1, 3, 4)
    )  # paired: [fp, r, 2, d, c]
    w2_t = np.ascontiguousarray(
        W2.reshape(FT, P, KD, P).transpose(2, 1, 0, 3)
    ).astype(ml_dtypes.bfloat16)  # w2[m, r, f, c] = W2[f*128+r, m*128+c]

    ppvec = np.zeros((P, 80), np.float32)
    for base, vec in [
        (PP_BO, bo), (PP_G1, g1), (PP_BE1, be1), (PP_B2, b2), (PP_G2, g2), (PP_BE2, be2),
    ]:
        ppvec[:, base : base + KD] = np.asarray(vec, np.float32).reshape(KD, P).T
    ppvec[:, PP_B1 : PP_B1 + FT] = np.asarray(b1, np.float32).reshape(FT, P).T

    iota = np.arange(512)
    masks = np.zeros((4, P, 512), np.float32)
    for jj in range(4):
        masks[jj] = (iota[None, :] >= (P * jj + np.arange(P))[:, None]).astype(np.float32)
    masks = np.ascontiguousarray(masks.transpose(1, 0, 2))  # [P, 4, 512]
    ones2 = np.ones((P, 2), np.float32)

    in_maps = []
    for c in range(NC_N):
        b_i, hh = c // 2, c % 2
        heads = slice(hh * 8, hh * 8 + 8)
        xT_c = round_f32r(
            np.ascontiguousarray(x[b_i].T.reshape(KD, P, S).transpose(1, 0, 2))
        )
        own = np.r_[hh * 512 : hh * 512 + 512, 1024 + hh * 512 : 1024 + hh * 512 + 512]
        x_resid_c = np.ascontiguousarray(x[b_i][own].T.reshape(KD, P, TOK))

        Wq8 = Wq[heads].reshape(8, KD, P, HD)  # [h, d, r, e]
        Wk8 = Wk[heads].reshape(8, KD, P, HD)
        Wv8 = Wv[heads]  # [8, D, HD]
        wq_c = np.empty((4, P, KD, P), np.float32)
        wk_c = np.empty((4, P, KD, P), np.float32)
        for p_i in range(4):
            for e in range(2):
                h = 2 * p_i + e
                wq_c[p_i, :, :, e * 64 : (e + 1) * 64] = Wq8[h].transpose(1, 0, 2)
                wk_c[p_i, :, :, e * 64 : (e + 1) * 64] = Wk8[h].transpose(1, 0, 2)
        wv_c = np.ascontiguousarray(
            Wv8.reshape(8, KD, P, HD).transpose(2, 1, 0, 3).reshape(P, KD, 8 * HD)
        )  # wv[r, d, h*64+e] = Wv8[h, d*128+r, e]
        Wo_own = Wo[hh * 512 : (hh + 1) * 512]  # [512, D]
        wo_c = np.ascontiguousarray(
            Wo_own.reshape(4, P, KD, P).transpose(2, 1, 0, 3)
        ).astype(ml_dtypes.bfloat16)  # wo[m, r, kp, c] = Wo_own[kp*128+r, m*128+c]

        bq8 = np.asarray(bq_, np.float32)[heads].reshape(4, P)
        bk8 = np.asarray(bk_, np.float32)[heads].reshape(4, P)
        bqk_c = np.concatenate([bq8.T, bk8.T], axis=1)  # [P, 8]
        bv8 = np.asarray(bv_, np.float32)[heads]

        in_maps.append(
            {
                "xT": xT_c,
                "x_resid": x_resid_c,
                "wq": round_f32r(wq_c),
                "wk": round_f32r(wk_c),
                "wv": round_f32r(wv_c),
                "wo": wo_c,
                "w1": w1_t,
                "w2": w2_t,
                "bqk": bqk_c,
                "bv_row": bv8.reshape(1, 8 * HD),
                "ppvec": ppvec,
                "masks": masks.astype(ml_dtypes.bfloat16),
                "ones2": ones2,
                "salt": np.full((1, 7), 12.0, np.float32),
            }
        )
    return in_maps


_NC_CACHE = {}


def _get_nc(ar_bypass=False):
    key = bool(ar_bypass)
    if key not in _NC_CACHE:
        _NC_CACHE[key] = build_nc(ar_bypass)
    return _NC_CACHE[key]


def assemble(results):
    out = np.empty((B, S, D), np.float32)
    for c in range(NC_N):
        b_i, hh = c // 2, c % 2
        own = np.r_[hh * 512 : hh * 512 + 512, 1024 + hh * 512 : 1024 + hh * 512 + 512]
        oT = results[c]["out"].reshape(D, TOK)
        out[b_i, own, :] = oT.T
    return out


def kernel(**inputs) -> np.ndarray:
    nc = _get_nc()
    in_maps = shard_inputs(
        inputs["x"], inputs["Wq"], inputs["bq"], inputs["Wk"], inputs["bk"],
        inputs["Wv"], inputs["bv"], inputs["Wo"], inputs["bo"],
        inputs["W1"], inputs["b1"], inputs["W2"], inputs["b2"],
        inputs["g1"], inputs["be1"], inputs["g2"], inputs["be2"],
    )
    res = run_bass_kernel_spmd(nc, in_maps, list(range(NC_N)))
    return assemble(res.results)
